# revision 23
# baseline (speedup 1.0000x reference)
"""Trainium2 Bass kernel for nn_Encoder_37967510897422 (GCN-GRU encoder).

Math (reference):
  per step t:  xh = [x_t, h]
               zr = sigmoid(gconv(xh, A) @ W_zr + b_zr); z, r = split(zr)
               xc = [x_t, r*h]
               c  = tanh(gconv(xc, A) @ W_c + b_c)
               h  = z*h + (1-z)*c
  gconv(u, A) = concat([u, hop1, hop2]), hop_k = a*u + (1-a)*A@hop_{k-1}

Kernel reformulation (per core, data-parallel over batch, Bs=4):
  gconv(u) @ W  =  u @ P + (A u) @ Q + (A^2 u) @ R      (hop algebra folded into
     P = W0 + a(W1+W2), Q = (1-a)(W1 + a W2), R = (1-a)^2 W2, host-side)
  x-dependent terms are precomputed per t ("pre" tiles, incl. bias); the
  recurrent part applies A / A^2 to h and r*h with node-major layouts, and
  gate projections use block-diagonal [2b x 2b] duplicated weights so each
  matmul runs with full K=128 / M=128 at 512 moving columns (f32r full rate).
All matmuls use float32r (fp32 with 11-bit mantissa, 1 cycle/row on PE).
"""

import numpy as np

import concourse.bacc as bacc
import concourse.bass as bass
import concourse.mybir as mybir
from concourse.tile import TileContext
from concourse.bass_utils import run_bass_kernel_spmd

F32 = mybir.dt.float32
F32R = mybir.dt.float32r
AF = mybir.ActivationFunctionType

B, N, T, D, H = 32, 512, 24, 64, 64
ALPHA = 0.05
NCORES = 8
BS = B // NCORES  # 4 batches per core
NBLK = N // 128  # 4


def round_f32r(a: np.ndarray) -> np.ndarray:
    """Round fp32 -> fp32r (11-bit mantissa, round-to-nearest-even)."""
    a = np.ascontiguousarray(a, dtype=np.float32)
    bits = a.view(np.uint32)
    lsb = (bits >> 12) & 1
    rounded = bits + 0x7FF + lsb
    rounded &= np.uint32(0xFFFFF000)
    return rounded.view(np.float32)


def build(nc: bass.Bass, n_steps: int = T):
    """Emit the Tile program for one core (Bs=4 batches)."""
    # ---------------- DRAM parameters ----------------
    at_d = nc.declare_dram_parameter("at", [N, N], F32R, isOutput=False)     # A^T (f32r)
    a2t_d = nc.declare_dram_parameter("a2t", [N, N], F32R, isOutput=False)   # (A^2)^T (f32r)
    bd_d = nc.declare_dram_parameter("bd", [18, 128, 128], F32R, isOutput=False)
    bias_d = nc.declare_dram_parameter("bias", [3, 128], F32, isOutput=False)
    eye_d = nc.declare_dram_parameter("eye", [128, 128], F32R, isOutput=False)
    x_d = nc.declare_dram_parameter("x", [BS, N, n_steps, D], F32, isOutput=False)
    xr_d = nc.declare_dram_parameter("xr", [BS, N, n_steps, D], F32R, isOutput=False)
    xrt_d = nc.declare_dram_parameter("xrt", [BS, n_steps, D, N], F32R, isOutput=False)
    out_d = nc.declare_dram_parameter("out", [BS, N, n_steps, H], F32, isOutput=True)
    out2_d = nc.declare_dram_parameter("out2", [BS, N, n_steps, H], F32, isOutput=True)

    # weight index in bd_d: [g(3: z,r,c), tau(3), side(2: x,h)]
    def wi(g, tau, side):
        return (g * 3 + tau) * 2 + side

    GZ, GR, GC = 0, 1, 2
    SX, SH = 0, 1

    with TileContext(nc) as tc:
        with (
            tc.tile_pool(name="const", bufs=1) as const,
            tc.tile_pool(name="xin", bufs=3) as xin,
            tc.tile_pool(name="xfp", bufs=4) as xfp,
            tc.tile_pool(name="xhop", bufs=4) as xhop,
            tc.tile_pool(name="state", bufs=2) as state,
            tc.tile_pool(name="hop", bufs=2) as hop,
            tc.tile_pool(name="gate", bufs=2) as gatep,
            tc.tile_pool(name="tmp", bufs=2) as tmpp,
            tc.tile_pool(name="outp", bufs=2) as outp,
            tc.tile_pool(name="ps", bufs=1, space="PSUM") as ps,
            tc.tile_pool(name="psa", bufs=4, space="PSUM") as psa,
            tc.tile_pool(name="psg", bufs=3, space="PSUM") as psg,
        ):
            # ---------------- constants ----------------
            # spread across HWDGE queues (vector/scalar/tensor) so the first
            # x-loads + matmuls aren't serialized behind 2.6 MB of constants
            at_sb = const.tile([128, NBLK, N], F32R, tag="at")
            a2t_sb = const.tile([128, NBLK, N], F32R, tag="a2t")
            for k in range(NBLK):
                nsl = slice(128 * k, 128 * (k + 1))
                nc.scalar.dma_start(out=at_sb[:, k], in_=at_d[nsl, :])
                nc.gpsimd.dma_start(out=a2t_sb[:, k], in_=a2t_d[nsl, :])
            bd_sb = const.tile([128, 18, 128], F32R, tag="bd")
            nc.scalar.dma_start(out=bd_sb, in_=bd_d.rearrange("w k m -> k w m"))
            bias_sb = const.tile([128, 3], F32, tag="bias")
            nc.scalar.dma_start(out=bias_sb, in_=bias_d.rearrange("g p -> p g"))
            eye_sb = const.tile([128, 128], F32R, tag="eye")
            nc.scalar.dma_start(out=eye_sb, in_=eye_d[:, :])

            def bd_w(g, tau, side):
                return bd_sb[:, wi(g, tau, side), :]

            # ---------------- precompute: x-projections for step t --------

            def precompute(t):
                pre_tiles = {}  # (g, pair) -> tile
                # node-major x_t (f32r) for A-matmul lhsT
                xrk = xin.tile([128, NBLK, BS, D], F32R, tag="xrk")
                xf = xfp.tile([128, NBLK, BS, D], F32, tag="xf")
                for k in range(NBLK):
                    nsl = slice(128 * k, 128 * (k + 1))
                    nc.sync.dma_start(
                        out=xrk[:, k], in_=xr_d[:, nsl, t, :].rearrange("b p d -> p b d")
                    )
                for k in range(NBLK):
                    nsl = slice(128 * k, 128 * (k + 1))
                    # plain-f32 x_t for the skip connection (needed later than xrk)
                    nc.sync.dma_start(
                        out=xf[:, k], in_=x_d[:, nsl, t, :].rearrange("b p d -> p b d")
                    )
                for pair in range(2):
                    bsl = slice(2 * pair, 2 * pair + 2)
                    # X^T pair tile [ (2b*64d), n ] loaded directly (host pre-transposed)
                    xt = xhop.tile([128, N], F32R, tag=f"xt{pair}")
                    for j in range(2):
                        nc.sync.dma_start(
                            out=xt[64 * j:64 * (j + 1), :],
                            in_=xrt_d[2 * pair + j, t, :, :],
                        )
                    # (A x)^T, (A^2 x)^T
                    pax = psa.tile([128, N], F32, tag="amm")
                    for k in range(NBLK):
                        nc.tensor.matmul(
                            pax,
                            xrk[:, k, bsl, :].rearrange("p b d -> p (b d)"),
                            at_sb[:, k, :],
                            start=(k == 0),
                            stop=(k == NBLK - 1),
                        )
                    axt = xhop.tile([128, N], F32R, tag=f"axt{pair}")
                    nc.scalar.copy(out=axt, in_=pax)
                    pa2x = psa.tile([128, N], F32, tag="amm")
                    for k in range(NBLK):
                        nc.tensor.matmul(
                            pa2x,
                            xrk[:, k, bsl, :].rearrange("p b d -> p (b d)"),
                            a2t_sb[:, k, :],
                            start=(k == 0),
                            stop=(k == NBLK - 1),
                        )
                    a2xt = xhop.tile([128, N], F32R, tag=f"a2xt{pair}")
                    nc.scalar.copy(out=a2xt, in_=pa2x)
                    pre_tiles[pair] = (xt, axt, a2xt)
                return xf, pre_tiles

            # ---------------- recurrent state ----------------
            h_T = []     # per pair: [ (2b*64h), n ] f32r
            h_node = [None, None]  # per pair: [128n, k, (2b*64h)] f32r
            zero_sb = const.tile([128, N], F32, tag="zero")
            nc.vector.memset(zero_sb, 0.0)
            for pair in range(2):
                ht = state.tile([128, N], F32R, tag=f"hT{pair}")
                nc.vector.tensor_copy(out=ht, in_=zero_sb)
                h_T.append(ht)

            pre_by_t = [precompute(0)]

            def a_chain(node_tile, which, pair):
                """A/A^2 application: out (A v)^T pair tile in SBUF (f32r)."""
                rhs_all = at_sb if which == 0 else a2t_sb
                pm = psa.tile([128, N], F32, tag="amm")
                for k in range(NBLK):
                    nc.tensor.matmul(
                        pm, node_tile[:, k, :], rhs_all[:, k, :],
                        start=(k == 0), stop=(k == NBLK - 1),
                    )
                sb = hop.tile([128, N], F32R, tag=f"hop{which}_{pair}")
                nc.scalar.copy(out=sb, in_=pm)
                return sb

            def transpose_group(src, pair, tag):
                """[ (2b*64), n ] -> node-major [128n, k, 128(2b*64)] f32r."""
                pt = ps.tile([128, NBLK, 128], F32R, tag="tr")
                for k in range(NBLK):
                    nc.tensor.transpose(
                        pt[:, k, :], src[:, 128 * k:128 * (k + 1)], eye_sb
                    )
                node = state.tile([128, NBLK, 128], F32R, tag=f"{tag}{pair}")
                nc.vector.tensor_copy(out=node, in_=pt)
                return node

            for t in range(n_steps):
                if t + 1 < n_steps:
                    pre_by_t.append(precompute(t + 1))
                xf, pre_tiles = pre_by_t[t]
                for pair in range(2):
                    bsl = slice(2 * pair, 2 * pair + 2)
                    ht = h_T[pair]
                    xt, axt, a2xt = pre_tiles[pair]
                    if t > 0:
                        ah = a_chain(h_node[pair], 0, pair)
                        a2h = a_chain(h_node[pair], 1, pair)
                    # gates r, z  (x-hop terms first: they can issue early)
                    gates = {}
                    for g in (GR, GZ):
                        pg = psg.tile([128, N], F32, tag="gate")
                        nc.tensor.matmul(pg, bd_w(g, 0, SX), xt, start=True, stop=False)
                        nc.tensor.matmul(pg, bd_w(g, 1, SX), axt, start=False, stop=False)
                        nc.tensor.matmul(pg, bd_w(g, 2, SX), a2xt, start=False, stop=False)
                        if t > 0:
                            nc.tensor.matmul(pg, bd_w(g, 0, SH), ht, start=False, stop=False)
                            nc.tensor.matmul(pg, bd_w(g, 1, SH), ah, start=False, stop=False)
                            nc.tensor.matmul(pg, bd_w(g, 2, SH), a2h, start=False, stop=True)
                        else:
                            nc.tensor.matmul(pg, bd_w(g, 0, SH), ht, start=False, stop=True)
                        gsb = gatep.tile([128, N], F32R, tag=f"g{g}_{pair}")
                        nc.scalar.activation(out=gsb, in_=pg, func=AF.Sigmoid,
                                             bias=bias_sb[:, g:g + 1], scale=1.0)
                        gates[g] = gsb
                    # rh = r * h
                    rh = tmpp.tile([128, N], F32R, tag=f"rh{pair}")
                    nc.vector.tensor_mul(rh, gates[GR], ht)
                    # c gate
                    pc = psg.tile([128, N], F32, tag="gate")
                    nc.tensor.matmul(pc, bd_w(GC, 0, SX), xt, start=True, stop=False)
                    nc.tensor.matmul(pc, bd_w(GC, 1, SX), axt, start=False, stop=False)
                    nc.tensor.matmul(pc, bd_w(GC, 2, SX), a2xt, start=False, stop=False)
                    if t > 0:
                        rh_node = transpose_group(rh, pair, "rhn")
                        arh = a_chain(rh_node, 0, pair)
                        a2rh = a_chain(rh_node, 1, pair)
                        nc.tensor.matmul(pc, bd_w(GC, 0, SH), rh, start=False, stop=False)
                        nc.tensor.matmul(pc, bd_w(GC, 1, SH), arh, start=False, stop=False)
                        nc.tensor.matmul(pc, bd_w(GC, 2, SH), a2rh, start=False, stop=True)
                    else:
                        nc.tensor.matmul(pc, bd_w(GC, 0, SH), rh, start=False, stop=True)
                    csb = gatep.tile([128, N], F32R, tag=f"gc_{pair}")
                    nc.scalar.activation(out=csb, in_=pc, func=AF.Tanh,
                                         bias=bias_sb[:, GC:GC + 1], scale=1.0)
                    # h' = c + z*(h - c)
                    u = tmpp.tile([128, N], F32R, tag=f"u{pair}")
                    nc.vector.tensor_sub(u, ht, csb)
                    v = tmpp.tile([128, N], F32R, tag=f"v{pair}")
                    nc.vector.tensor_mul(v, gates[GZ], u)
                    ht_new = state.tile([128, N], F32R, tag=f"hT{pair}")
                    nc.vector.tensor_add(ht_new, csb, v)
                    # node-major h' (for next step's A-matmuls + outputs)
                    hn = transpose_group(ht_new, pair, "hn")
                    # outputs
                    o2 = outp.tile([128, NBLK, 2, D], F32, tag=f"o2_{pair}")
                    nc.vector.tensor_add(
                        o2,
                        hn.rearrange("p k (b d) -> p k b d", b=2).bitcast(F32),
                        xf[:, :, bsl, :],
                    )
                    for k in range(NBLK):
                        nsl = slice(128 * k, 128 * (k + 1))
                        nc.gpsimd.dma_start(
                            out=out_d[bsl, nsl, t, :].rearrange("b p d -> p b d"),
                            in_=hn[:, k].rearrange("p (b d) -> p b d", b=2).bitcast(F32),
                        )
                        nc.gpsimd.dma_start(
                            out=out2_d[bsl, nsl, t, :].rearrange("b p d -> p b d"),
                            in_=o2[:, k],
                        )
                    h_T[pair] = ht_new
                    h_node[pair] = hn
    return nc


def _host_prep(x, adjs, W_zr, b_zr, W_c, b_c, n_steps=T):
    """Build all host-side constant arrays + per-core input maps."""
    x = np.asarray(x, dtype=np.float32)
    A = np.asarray(adjs, dtype=np.float32)
    W_zr = np.asarray(W_zr, dtype=np.float32)
    W_c = np.asarray(W_c, dtype=np.float32)
    b_zr = np.asarray(b_zr, dtype=np.float32)
    b_c = np.asarray(b_c, dtype=np.float32)

    A2 = (A.astype(np.float64) @ A.astype(np.float64)).astype(np.float32)
    at = round_f32r(np.ascontiguousarray(A.T))
    a2t = round_f32r(np.ascontiguousarray(A2.T))

    W = np.concatenate([W_zr, W_c], axis=1)  # [384, 192]
    W0, W1, W2 = W[0:128], W[128:256], W[256:384]
    Pm = W0 + ALPHA * (W1 + W2)
    Qm = (1.0 - ALPHA) * (W1 + ALPHA * W2)
    Rm = (1.0 - ALPHA) ** 2 * W2
    # gate column groups in W-concat: z: 0:64, r: 64:128, c: 128:192
    gcols = {0: (0, 64), 1: (64, 128), 2: (128, 192)}
    bd = np.zeros((18, 128, 128), np.float32)
    for g in range(3):
        c0, c1 = gcols[g]
        for tau, M in enumerate((Pm, Qm, Rm)):
            for side in range(2):  # 0: x rows 0:64, 1: h rows 64:128
                w = M[64 * side:64 * side + 64, c0:c1]
                blk = np.zeros((128, 128), np.float32)
                blk[0:64, 0:64] = w
                blk[64:128, 64:128] = w
                bd[(g * 3 + tau) * 2 + side] = blk
    bd = round_f32r(bd)

    bias = np.zeros((3, 128), np.float32)
    bias[0] = np.tile(b_zr[0:64], 2)
    bias[1] = np.tile(b_zr[64:128], 2)
    bias[2] = np.tile(b_c, 2)

    eye = round_f32r(np.eye(128, dtype=np.float32))

    xr = round_f32r(x[:, :, :n_steps, :])
    xrt = np.ascontiguousarray(xr.transpose(0, 2, 3, 1))  # [B, T, D, N]

    common = {"at": at, "a2t": a2t, "bd": bd, "bias": bias, "eye": eye}
    in_maps = []
    for c in range(NCORES):
        bsl = slice(c * BS, (c + 1) * BS)
        in_maps.append({
            **common,
            "x": np.ascontiguousarray(x[bsl, :, :n_steps, :]),
            "xr": np.ascontiguousarray(xr[bsl]),
            "xrt": np.ascontiguousarray(xrt[bsl]),
        })
    return in_maps


_CACHE = {}


def _get_nc(n_steps=T):
    if n_steps not in _CACHE:
        nc = bacc.Bacc("TRN2")
        build(nc, n_steps)
        nc.finalize()
        _CACHE[n_steps] = nc
    return _CACHE[n_steps]


def kernel(x, seq_length, adjs, W_zr, b_zr, W_c, b_c):
    x = np.asarray(x, dtype=np.float32)
    A = np.asarray(adjs, dtype=np.float32)
    n_steps = T
    nc = _get_nc(n_steps)
    in_maps = _host_prep(x, A, W_zr, b_zr, W_c, b_c, n_steps)
    res = run_bass_kernel_spmd(nc, in_maps, core_ids=list(range(NCORES)))
    outputs = np.concatenate([r["out"] for r in res.results], axis=0)
    outputs_2 = np.concatenate([r["out2"] for r in res.results], axis=0)
    hiddens = outputs[:, None, :, :, :]
    adjs_output = np.broadcast_to(A, (n_steps, N, N))
    return (outputs_2, hiddens, adjs_output)


# revision 24
# speedup vs baseline: 1.0346x; 1.0346x over previous
"""Trainium2 Bass kernel for nn_Encoder_37967510897422 (GCN-GRU encoder).

Math (reference):
  per step t:  xh = [x_t, h]
               zr = sigmoid(gconv(xh, A) @ W_zr + b_zr); z, r = split(zr)
               xc = [x_t, r*h]
               c  = tanh(gconv(xc, A) @ W_c + b_c)
               h  = z*h + (1-z)*c
  gconv(u, A) = concat([u, hop1, hop2]), hop_k = a*u + (1-a)*A@hop_{k-1}

Kernel reformulation (per core, data-parallel over batch, Bs=4):
  gconv(u) @ W  =  u @ P + (A u) @ Q + (A^2 u) @ R      (hop algebra folded into
     P = W0 + a(W1+W2), Q = (1-a)(W1 + a W2), R = (1-a)^2 W2, host-side)
  x-dependent terms are precomputed per t ("pre" tiles, incl. bias); the
  recurrent part applies A / A^2 to h and r*h with node-major layouts, and
  gate projections use block-diagonal [2b x 2b] duplicated weights so each
  matmul runs with full K=128 / M=128 at 512 moving columns (f32r full rate).
All matmuls use float32r (fp32 with 11-bit mantissa, 1 cycle/row on PE).
"""

import numpy as np

import concourse.bacc as bacc
import concourse.bass as bass
import concourse.mybir as mybir
from concourse.tile import TileContext
from concourse.bass_utils import run_bass_kernel_spmd

F32 = mybir.dt.float32
F32R = mybir.dt.float32r
AF = mybir.ActivationFunctionType

B, N, T, D, H = 32, 512, 24, 64, 64
ALPHA = 0.05
NCORES = 8
BS = B // NCORES  # 4 batches per core
NBLK = N // 128  # 4


def round_f32r(a: np.ndarray) -> np.ndarray:
    """Round fp32 -> fp32r (11-bit mantissa, round-to-nearest-even)."""
    a = np.ascontiguousarray(a, dtype=np.float32)
    bits = a.view(np.uint32)
    lsb = (bits >> 12) & 1
    rounded = bits + 0x7FF + lsb
    rounded &= np.uint32(0xFFFFF000)
    return rounded.view(np.float32)


def build(nc: bass.Bass, n_steps: int = T):
    """Emit the Tile program for one core (Bs=4 batches)."""
    # ---------------- DRAM parameters ----------------
    at_d = nc.declare_dram_parameter("at", [N, N], F32R, isOutput=False)     # A^T (f32r)
    a2t_d = nc.declare_dram_parameter("a2t", [N, N], F32R, isOutput=False)   # (A^2)^T (f32r)
    bd_d = nc.declare_dram_parameter("bd", [18, 128, 128], F32R, isOutput=False)
    bias_d = nc.declare_dram_parameter("bias", [3, 128], F32, isOutput=False)
    eye_d = nc.declare_dram_parameter("eye", [128, 128], F32R, isOutput=False)
    x_d = nc.declare_dram_parameter("x", [BS, N, n_steps, D], F32, isOutput=False)
    xr_d = nc.declare_dram_parameter("xr", [BS, N, n_steps, D], F32R, isOutput=False)
    xrt_d = nc.declare_dram_parameter("xrt", [BS, n_steps, D, N], F32R, isOutput=False)
    out_d = nc.declare_dram_parameter("out", [BS, N, n_steps, H], F32, isOutput=True)
    out2_d = nc.declare_dram_parameter("out2", [BS, N, n_steps, H], F32, isOutput=True)

    # weight index in bd_d: [g(3: z,r,c), tau(3), side(2: x,h)]
    def wi(g, tau, side):
        return (g * 3 + tau) * 2 + side

    GZ, GR, GC = 0, 1, 2
    SX, SH = 0, 1

    with TileContext(nc) as tc:
        with (
            tc.tile_pool(name="const", bufs=1) as const,
            tc.tile_pool(name="xin", bufs=3) as xin,
            tc.tile_pool(name="xfp", bufs=4) as xfp,
            tc.tile_pool(name="xhop", bufs=4) as xhop,
            tc.tile_pool(name="state", bufs=2) as state,
            tc.tile_pool(name="hop", bufs=2) as hop,
            tc.tile_pool(name="gate", bufs=2) as gatep,
            tc.tile_pool(name="tmp", bufs=2) as tmpp,
            tc.tile_pool(name="outp", bufs=2) as outp,
            tc.tile_pool(name="ps", bufs=2, space="PSUM") as ps,
            tc.tile_pool(name="psa", bufs=3, space="PSUM") as psa,
            tc.tile_pool(name="psg", bufs=3, space="PSUM") as psg,
        ):
            # ---------------- constants ----------------
            # spread across HWDGE queues (vector/scalar/tensor) so the first
            # x-loads + matmuls aren't serialized behind 2.6 MB of constants
            at_sb = const.tile([128, NBLK, N], F32R, tag="at")
            a2t_sb = const.tile([128, NBLK, N], F32R, tag="a2t")
            for k in range(NBLK):
                nsl = slice(128 * k, 128 * (k + 1))
                nc.scalar.dma_start(out=at_sb[:, k], in_=at_d[nsl, :])
                nc.gpsimd.dma_start(out=a2t_sb[:, k], in_=a2t_d[nsl, :])
            bd_sb = const.tile([128, 18, 128], F32R, tag="bd")
            nc.scalar.dma_start(out=bd_sb, in_=bd_d.rearrange("w k m -> k w m"))
            bias_sb = const.tile([128, 3], F32, tag="bias")
            nc.scalar.dma_start(out=bias_sb, in_=bias_d.rearrange("g p -> p g"))
            eye_sb = const.tile([128, 128], F32R, tag="eye")
            nc.scalar.dma_start(out=eye_sb, in_=eye_d[:, :])

            def bd_w(g, tau, side):
                return bd_sb[:, wi(g, tau, side), :]

            # ---------------- precompute: x-projections for step t --------

            def precompute(t):
                pre_tiles = {}  # (g, pair) -> tile
                # node-major x_t (f32r) for A-matmul lhsT
                xrk = xin.tile([128, NBLK, BS, D], F32R, tag="xrk")
                xf = xfp.tile([128, NBLK, BS, D], F32, tag="xf")
                for k in range(NBLK):
                    nsl = slice(128 * k, 128 * (k + 1))
                    nc.sync.dma_start(
                        out=xrk[:, k], in_=xr_d[:, nsl, t, :].rearrange("b p d -> p b d")
                    )
                for k in range(NBLK):
                    nsl = slice(128 * k, 128 * (k + 1))
                    # plain-f32 x_t for the skip connection (needed later than xrk)
                    nc.sync.dma_start(
                        out=xf[:, k], in_=x_d[:, nsl, t, :].rearrange("b p d -> p b d")
                    )
                for pair in range(2):
                    bsl = slice(2 * pair, 2 * pair + 2)
                    # X^T pair tile [ (2b*64d), n ] loaded directly (host pre-transposed)
                    xt = xhop.tile([128, N], F32R, tag=f"xt{pair}")
                    for j in range(2):
                        nc.sync.dma_start(
                            out=xt[64 * j:64 * (j + 1), :],
                            in_=xrt_d[2 * pair + j, t, :, :],
                        )
                    # (A x)^T, (A^2 x)^T
                    pax = psa.tile([128, N], F32, tag="amm")
                    for k in range(NBLK):
                        nc.tensor.matmul(
                            pax,
                            xrk[:, k, bsl, :].rearrange("p b d -> p (b d)"),
                            at_sb[:, k, :],
                            start=(k == 0),
                            stop=(k == NBLK - 1),
                        )
                    axt = xhop.tile([128, N], F32R, tag=f"axt{pair}")
                    nc.scalar.copy(out=axt, in_=pax)
                    pa2x = psa.tile([128, N], F32, tag="amm")
                    for k in range(NBLK):
                        nc.tensor.matmul(
                            pa2x,
                            xrk[:, k, bsl, :].rearrange("p b d -> p (b d)"),
                            a2t_sb[:, k, :],
                            start=(k == 0),
                            stop=(k == NBLK - 1),
                        )
                    a2xt = xhop.tile([128, N], F32R, tag=f"a2xt{pair}")
                    nc.scalar.copy(out=a2xt, in_=pa2x)
                    pre_tiles[pair] = (xt, axt, a2xt)
                return xf, pre_tiles

            # ---------------- recurrent state ----------------
            h_T = []     # per pair: [ (2b*64h), n ] f32r
            h_node = [None, None]  # per pair: [128n, k, (2b*64h)] f32r
            zero_sb = const.tile([128, N], F32, tag="zero")
            nc.vector.memset(zero_sb, 0.0)
            for pair in range(2):
                ht = state.tile([128, N], F32R, tag=f"hT{pair}")
                nc.vector.tensor_copy(out=ht, in_=zero_sb)
                h_T.append(ht)

            pre_by_t = [precompute(0)]

            def a_chain(node_tile, which, pair):
                """A/A^2 application: out (A v)^T pair tile in SBUF (f32r)."""
                rhs_all = at_sb if which == 0 else a2t_sb
                pm = psa.tile([128, N], F32, tag="amm")
                for k in range(NBLK):
                    nc.tensor.matmul(
                        pm, node_tile[:, k, :], rhs_all[:, k, :],
                        start=(k == 0), stop=(k == NBLK - 1),
                    )
                sb = hop.tile([128, N], F32R, tag=f"hop{which}_{pair}")
                nc.scalar.copy(out=sb, in_=pm)
                return sb

            def transpose_group(src, pair, tag):
                """[ (2b*64), n ] -> node-major [128n, k, 128(2b*64)] f32r."""
                pt = ps.tile([128, NBLK, 128], F32R, tag="tr")
                for k in range(NBLK):
                    nc.tensor.transpose(
                        pt[:, k, :], src[:, 128 * k:128 * (k + 1)], eye_sb
                    )
                node = state.tile([128, NBLK, 128], F32R, tag=f"{tag}{pair}")
                nc.vector.tensor_copy(out=node, in_=pt)
                return node

            for t in range(n_steps):
                if t + 1 < n_steps:
                    pre_by_t.append(precompute(t + 1))
                xf, pre_tiles = pre_by_t[t]
                for pair in range(2):
                    bsl = slice(2 * pair, 2 * pair + 2)
                    ht = h_T[pair]
                    xt, axt, a2xt = pre_tiles[pair]
                    if t > 0:
                        ah = a_chain(h_node[pair], 0, pair)
                        a2h = a_chain(h_node[pair], 1, pair)
                    # gates r, z  (x-hop terms first: they can issue early)
                    gates = {}
                    for g in (GR, GZ):
                        pg = psg.tile([128, N], F32, tag="gate")
                        nc.tensor.matmul(pg, bd_w(g, 0, SX), xt, start=True, stop=False)
                        nc.tensor.matmul(pg, bd_w(g, 1, SX), axt, start=False, stop=False)
                        nc.tensor.matmul(pg, bd_w(g, 2, SX), a2xt, start=False, stop=False)
                        if t > 0:
                            nc.tensor.matmul(pg, bd_w(g, 0, SH), ht, start=False, stop=False)
                            nc.tensor.matmul(pg, bd_w(g, 1, SH), ah, start=False, stop=False)
                            nc.tensor.matmul(pg, bd_w(g, 2, SH), a2h, start=False, stop=True)
                        else:
                            nc.tensor.matmul(pg, bd_w(g, 0, SH), ht, start=False, stop=True)
                        gsb = gatep.tile([128, N], F32R, tag=f"g{g}_{pair}")
                        nc.scalar.activation(out=gsb, in_=pg, func=AF.Sigmoid,
                                             bias=bias_sb[:, g:g + 1], scale=1.0)
                        gates[g] = gsb
                    # rh = r * h
                    rh = tmpp.tile([128, N], F32R, tag=f"rh{pair}")
                    nc.vector.tensor_mul(rh, gates[GR], ht)
                    # c gate
                    pc = psg.tile([128, N], F32, tag="gate")
                    nc.tensor.matmul(pc, bd_w(GC, 0, SX), xt, start=True, stop=False)
                    nc.tensor.matmul(pc, bd_w(GC, 1, SX), axt, start=False, stop=False)
                    nc.tensor.matmul(pc, bd_w(GC, 2, SX), a2xt, start=False, stop=False)
                    if t > 0:
                        rh_node = transpose_group(rh, pair, "rhn")
                        arh = a_chain(rh_node, 0, pair)
                        a2rh = a_chain(rh_node, 1, pair)
                        nc.tensor.matmul(pc, bd_w(GC, 0, SH), rh, start=False, stop=False)
                        nc.tensor.matmul(pc, bd_w(GC, 1, SH), arh, start=False, stop=False)
                        nc.tensor.matmul(pc, bd_w(GC, 2, SH), a2rh, start=False, stop=True)
                    else:
                        nc.tensor.matmul(pc, bd_w(GC, 0, SH), rh, start=False, stop=True)
                    csb = gatep.tile([128, N], F32R, tag=f"gc_{pair}")
                    nc.scalar.activation(out=csb, in_=pc, func=AF.Tanh,
                                         bias=bias_sb[:, GC:GC + 1], scale=1.0)
                    # h' = c + z*(h - c)
                    u = tmpp.tile([128, N], F32R, tag=f"u{pair}")
                    nc.vector.tensor_sub(u, ht, csb)
                    v = tmpp.tile([128, N], F32R, tag=f"v{pair}")
                    nc.vector.tensor_mul(v, gates[GZ], u)
                    ht_new = state.tile([128, N], F32R, tag=f"hT{pair}")
                    nc.vector.tensor_add(ht_new, csb, v)
                    # node-major h' (for next step's A-matmuls + outputs)
                    hn = transpose_group(ht_new, pair, "hn")
                    # outputs
                    o2 = outp.tile([128, NBLK, 2, D], F32, tag=f"o2_{pair}")
                    nc.vector.tensor_add(
                        o2,
                        hn.rearrange("p k (b d) -> p k b d", b=2).bitcast(F32),
                        xf[:, :, bsl, :],
                    )
                    for k in range(NBLK):
                        nsl = slice(128 * k, 128 * (k + 1))
                        nc.gpsimd.dma_start(
                            out=out_d[bsl, nsl, t, :].rearrange("b p d -> p b d"),
                            in_=hn[:, k].rearrange("p (b d) -> p b d", b=2).bitcast(F32),
                        )
                        nc.gpsimd.dma_start(
                            out=out2_d[bsl, nsl, t, :].rearrange("b p d -> p b d"),
                            in_=o2[:, k],
                        )
                    h_T[pair] = ht_new
                    h_node[pair] = hn
    return nc


def _host_prep(x, adjs, W_zr, b_zr, W_c, b_c, n_steps=T):
    """Build all host-side constant arrays + per-core input maps."""
    x = np.asarray(x, dtype=np.float32)
    A = np.asarray(adjs, dtype=np.float32)
    W_zr = np.asarray(W_zr, dtype=np.float32)
    W_c = np.asarray(W_c, dtype=np.float32)
    b_zr = np.asarray(b_zr, dtype=np.float32)
    b_c = np.asarray(b_c, dtype=np.float32)

    A2 = (A.astype(np.float64) @ A.astype(np.float64)).astype(np.float32)
    at = round_f32r(np.ascontiguousarray(A.T))
    a2t = round_f32r(np.ascontiguousarray(A2.T))

    W = np.concatenate([W_zr, W_c], axis=1)  # [384, 192]
    W0, W1, W2 = W[0:128], W[128:256], W[256:384]
    Pm = W0 + ALPHA * (W1 + W2)
    Qm = (1.0 - ALPHA) * (W1 + ALPHA * W2)
    Rm = (1.0 - ALPHA) ** 2 * W2
    # gate column groups in W-concat: z: 0:64, r: 64:128, c: 128:192
    gcols = {0: (0, 64), 1: (64, 128), 2: (128, 192)}
    bd = np.zeros((18, 128, 128), np.float32)
    for g in range(3):
        c0, c1 = gcols[g]
        for tau, M in enumerate((Pm, Qm, Rm)):
            for side in range(2):  # 0: x rows 0:64, 1: h rows 64:128
                w = M[64 * side:64 * side + 64, c0:c1]
                blk = np.zeros((128, 128), np.float32)
                blk[0:64, 0:64] = w
                blk[64:128, 64:128] = w
                bd[(g * 3 + tau) * 2 + side] = blk
    bd = round_f32r(bd)

    bias = np.zeros((3, 128), np.float32)
    bias[0] = np.tile(b_zr[0:64], 2)
    bias[1] = np.tile(b_zr[64:128], 2)
    bias[2] = np.tile(b_c, 2)

    eye = round_f32r(np.eye(128, dtype=np.float32))

    xr = round_f32r(x[:, :, :n_steps, :])
    xrt = np.ascontiguousarray(xr.transpose(0, 2, 3, 1))  # [B, T, D, N]

    common = {"at": at, "a2t": a2t, "bd": bd, "bias": bias, "eye": eye}
    in_maps = []
    for c in range(NCORES):
        bsl = slice(c * BS, (c + 1) * BS)
        in_maps.append({
            **common,
            "x": np.ascontiguousarray(x[bsl, :, :n_steps, :]),
            "xr": np.ascontiguousarray(xr[bsl]),
            "xrt": np.ascontiguousarray(xrt[bsl]),
        })
    return in_maps


_CACHE = {}


def _get_nc(n_steps=T):
    if n_steps not in _CACHE:
        nc = bacc.Bacc("TRN2")
        build(nc, n_steps)
        nc.finalize()
        _CACHE[n_steps] = nc
    return _CACHE[n_steps]


def kernel(x, seq_length, adjs, W_zr, b_zr, W_c, b_c):
    x = np.asarray(x, dtype=np.float32)
    A = np.asarray(adjs, dtype=np.float32)
    n_steps = T
    nc = _get_nc(n_steps)
    in_maps = _host_prep(x, A, W_zr, b_zr, W_c, b_c, n_steps)
    res = run_bass_kernel_spmd(nc, in_maps, core_ids=list(range(NCORES)))
    outputs = np.concatenate([r["out"] for r in res.results], axis=0)
    outputs_2 = np.concatenate([r["out2"] for r in res.results], axis=0)
    hiddens = outputs[:, None, :, :, :]
    adjs_output = np.broadcast_to(A, (n_steps, N, N))
    return (outputs_2, hiddens, adjs_output)
